# revision 9
# baseline (speedup 1.0000x reference)
"""Trainium2 Bass kernel for DetectionLoss (L1 + GIoU + CAM supervision).

Contract: kernel(**inputs) takes the FULL unsharded inputs (numpy arrays,
keyed as in setup_inputs()) and returns the FULL output, a float32 [4] array
[loss_l1, loss_giou, loss_cam, loss_total].

Sharding: data-parallel over batch B=16 across 8 NeuronCores (2 batches per
core). Positives (pos_*) are routed host-side to the core that owns their
batch, and gather indices (pure index arithmetic) are precomputed host-side
as part of the routing. Each core gathers only the CAM channel maps and
positive boxes it needs via indirect DMA (~1MB instead of ~13MB per core),
computes partial sums (l1, giou*w, cam-term) on device, and the host adds
the 8 partial vectors and applies the final scaling.

Layout: each needed CAM channel map [64,64] is spread over 2 SBUF partitions
(2048 elements each), and each partition row is gathered as two 4KB
quarter-rows so the masked reduction can start on the first half while the
second half is still in flight.
"""

import numpy as np

# Problem constants (hardcoded per the task contract).
B, C, H, W, K = 16, 80, 64, 64, 32
NCORES = 8
BPC = B // NCORES           # batches per core = 2
PAIRS = BPC * K             # CAM (box,channel) pairs per core = 64
HW = H * W                  # 4096
HALF = HW // 2              # 2048 elements per partition row
QTR = HW // 4               # 1024 elements per gathered quarter-row
LAMBDA_L1, LAMBDA_GIOU, LAMBDA_CAM = 1.0, 2.0, 0.5
EPS = 1e-6

# fpack column layout (single packed f32 [128, FCOLS] input per core)
_KV0 = 0            # kvals: 1..63            -> cols [0, 63)
_CV0 = 63           # colvals: 0..63          -> cols [63, 127)
_RV0 = 127          # rowvals (per half)      -> cols [127, 159)
_GT0 = 159          # gt box coords (dup)     -> cols [159, 163)
_PW = 163           # positive slot weight
_HT = 164           # half tag: 1.0 for partitions < 64, else 0.0
FCOLS = 165

# ipack columns (int32 [128, 4]): gather indices, host-precomputed routing
_I_CAMA = 0         # quarter-row A into cam4 [640, 1024]
_I_CAMB = 1         # quarter-row B into cam4 [640, 1024]
_I_PRED = 2         # row into pred [BPC*C*H*W, 4]
_I_GT = 3           # row into gtb [64, 4]
ICOLS = 4


def _build_kernel(debug=False, nslot=64):
    import concourse.bacc as bacc
    import concourse.mybir as mybir
    from concourse import bass
    from concourse.tile import TileContext

    f32 = mybir.dt.float32
    i32 = mybir.dt.int32
    Alu = mybir.AluOpType
    Act = mybir.ActivationFunctionType

    nc = bacc.Bacc("TRN2", target_bir_lowering=False, debug=False,
                   num_devices=NCORES)

    cam4 = nc.dram_tensor("cam4", [BPC * C * 4, QTR], f32, kind="ExternalInput")
    pred = nc.dram_tensor("pred", [BPC * C * H * W, 4], f32, kind="ExternalInput")
    gtb = nc.dram_tensor("gtb", [PAIRS, 4], f32, kind="ExternalInput")
    ipk = nc.dram_tensor("ipk", [128, ICOLS], i32, kind="ExternalInput")
    fpk = nc.dram_tensor("fpk", [128, FCOLS], f32, kind="ExternalInput")
    out = nc.dram_tensor("out", [4, 1], f32, kind="ExternalOutput")
    if debug:
        dbg = nc.dram_tensor("dbg", [128, 48], f32, kind="ExternalOutput")

    with TileContext(nc) as tc:
        with (
            tc.tile_pool(name="pool", bufs=1) as pool,
            tc.tile_pool(name="psum", bufs=1, space="PSUM") as pp,
        ):
            # ---- index load + gathers first: the CAM gather is the long pole
            IP = pool.tile([128, ICOLS], i32)
            nc.sync.dma_start(out=IP[:], in_=ipk.ap())
            CAM = pool.tile([128, HALF], f32)
            nc.gpsimd.indirect_dma_start(
                out=CAM[:, 0:QTR], out_offset=None, in_=cam4.ap(),
                in_offset=bass.IndirectOffsetOnAxis(
                    ap=IP[:, _I_CAMA:_I_CAMA + 1], axis=0))
            nc.gpsimd.indirect_dma_start(
                out=CAM[:, QTR:HALF], out_offset=None, in_=cam4.ap(),
                in_offset=bass.IndirectOffsetOnAxis(
                    ap=IP[:, _I_CAMB:_I_CAMB + 1], axis=0))
            PG = pool.tile([128, 8], f32)  # cols 0:4 pred box, 4:8 gt box
            nc.gpsimd.memset(PG[:], 0.0)
            nc.gpsimd.indirect_dma_start(
                out=PG[0:nslot, 0:4], out_offset=None, in_=pred.ap(),
                in_offset=bass.IndirectOffsetOnAxis(
                    ap=IP[0:nslot, _I_PRED:_I_PRED + 1], axis=0))
            nc.gpsimd.indirect_dma_start(
                out=PG[0:nslot, 4:8], out_offset=None, in_=gtb.ap(),
                in_offset=bass.IndirectOffsetOnAxis(
                    ap=IP[0:nslot, _I_GT:_I_GT + 1], axis=0))

            F = pool.tile([128, FCOLS], f32)
            nc.sync.dma_start(out=F[:], in_=fpk.ap())

            PART = pool.tile([128, 4], f32)
            nc.gpsimd.memset(PART[:], 0.0)
            ONES = pool.tile([128, 1], f32)
            nc.gpsimd.memset(ONES[:], 1.0)
            # difference of pred/gt boxes, feeds the ACT L1 pass
            D = pool.tile([128, 4], f32)
            nc.gpsimd.tensor_tensor(
                out=D[:], in0=PG[:, 0:4], in1=PG[:, 4:8], op=Alu.subtract)

            # ---- exact floor of 64*coords via comparison-sum ----
            SC = pool.tile([128, 4], f32)
            nc.vector.tensor_scalar_mul(SC[:], F[:, _GT0:_GT0 + 4], float(W))
            GEB = pool.tile([128, 4 * 63], f32)
            nc.vector.tensor_tensor(
                out=GEB[:].rearrange("p (c k) -> p c k", k=63),
                in0=SC[:].unsqueeze(2).to_broadcast([128, 4, 63]),
                in1=F[:, _KV0:_KV0 + 63].unsqueeze(1).to_broadcast([128, 4, 63]),
                op=Alu.is_ge)
            IC = pool.tile([128, 4], f32)  # jmin, imin, jmax, imax
            nc.vector.tensor_reduce(
                out=IC[:], in_=GEB[:].rearrange("p (c k) -> p c k", k=63),
                axis=mybir.AxisListType.X, op=Alu.add)

            # ---- row/col interval masks -> 2D mask (two column-halves) ----
            CGE = pool.tile([128, 64], f32)
            nc.vector.tensor_scalar(
                out=CGE[:], in0=F[:, _CV0:_CV0 + 64], scalar1=IC[:, 0:1],
                scalar2=None, op0=Alu.is_ge)
            CM = pool.tile([128, 64], f32)
            nc.vector.scalar_tensor_tensor(
                out=CM[:], in0=F[:, _CV0:_CV0 + 64], scalar=IC[:, 2:3],
                in1=CGE[:], op0=Alu.is_le, op1=Alu.mult)
            RGE = pool.tile([128, 32], f32)
            nc.vector.tensor_scalar(
                out=RGE[:], in0=F[:, _RV0:_RV0 + 32], scalar1=IC[:, 1:2],
                scalar2=None, op0=Alu.is_ge)
            RM = pool.tile([128, 32], f32)
            nc.vector.scalar_tensor_tensor(
                out=RM[:], in0=F[:, _RV0:_RV0 + 32], scalar=IC[:, 3:4],
                in1=RGE[:], op0=Alu.is_le, op1=Alu.mult)
            M2D = pool.tile([128, HALF], f32)
            nc.vector.tensor_tensor(
                out=M2D[:, 0:QTR].rearrange("p (h w) -> p h w", w=64),
                in0=RM[:, 0:16].unsqueeze(2).to_broadcast([128, 16, 64]),
                in1=CM[:].unsqueeze(1).to_broadcast([128, 16, 64]),
                op=Alu.mult)
            nc.vector.tensor_tensor(
                out=M2D[:, QTR:HALF].rearrange("p (h w) -> p h w", w=64),
                in0=RM[:, 16:32].unsqueeze(2).to_broadcast([128, 16, 64]),
                in1=CM[:].unsqueeze(1).to_broadcast([128, 16, 64]),
                op=Alu.mult)

            # ---- big masked reductions, split by gathered quarter ----
            # ST cols: 0 box_in qA, 1 box_in qB, 2 tot qA, 3 tot qB
            ST = pool.tile([128, 4], f32)
            MK = pool.tile([128, HALF], f32)
            nc.vector.scalar_tensor_tensor(
                out=MK[:, 0:QTR], in0=CAM[:, 0:QTR], scalar=1.0,
                in1=M2D[:, 0:QTR], op0=Alu.mult, op1=Alu.mult,
                accum_out=ST[:, 0:1])
            nc.vector.scalar_tensor_tensor(
                out=MK[:, QTR:HALF], in0=CAM[:, QTR:HALF], scalar=1.0,
                in1=M2D[:, QTR:HALF], op0=Alu.mult, op1=Alu.mult,
                accum_out=ST[:, 1:2])
            AO = pool.tile([128, HALF], f32)
            nc.scalar.activation(
                out=AO[:, 0:QTR], in_=CAM[:, 0:QTR], func=Act.Copy,
                accum_out=ST[:, 2:3])
            nc.scalar.activation(
                out=AO[:, QTR:HALF], in_=CAM[:, QTR:HALF], func=Act.Copy,
                accum_out=ST[:, 3:4])

            # ---- interval-count chain (independent of the CAM data) ----
            CNT = pool.tile([128, 2], f32)  # (jcnt, icnt)
            nc.vector.scalar_tensor_tensor(
                out=CNT[:], in0=IC[:, 2:4], scalar=1.0,
                in1=IC[:, 0:2], op0=Alu.add, op1=Alu.subtract)
            R2C = pool.tile([128, 2], f32)
            nc.vector.tensor_scalar_max(R2C[:], CNT[:], 0.0)
            SS = pool.tile([128, 2], f32)  # col0: s_in, col1: s_out
            nc.vector.tensor_tensor(
                out=SS[:, 0:1], in0=R2C[:, 0:1], in1=R2C[:, 1:2], op=Alu.mult)
            nc.vector.tensor_scalar(
                out=SS[:, 1:2], in0=SS[:, 0:1], scalar1=-1.0,
                scalar2=float(HW), op0=Alu.mult, op1=Alu.add)
            MM = pool.tile([128, 2], f32)
            nc.vector.tensor_scalar_max(MM[:], SS[:], 1.0)
            RR = pool.tile([128, 2], f32)
            nc.vector.reciprocal(RR[:], MM[:])
            G12 = pool.tile([128, 2], f32)
            nc.vector.tensor_scalar(
                out=G12[:], in0=SS[:], scalar1=0.0, scalar2=None,
                op0=Alu.is_gt)

            # ---- L1 (ACT: |d*w| with per-partition scale, accumulated) ----
            DABS = pool.tile([128, 4], f32)
            nc.scalar.activation(
                out=DABS[:], in_=D[:], func=Act.Abs,
                scale=F[:, _PW:_PW + 1], accum_out=PART[:, 0:1])

            # ---- GIoU on gathered positives (independent of CAM) ----
            MX = pool.tile([128, 4], f32)
            nc.vector.tensor_tensor(
                out=MX[:], in0=PG[:, 0:4], in1=PG[:, 4:8], op=Alu.max)
            MN = pool.tile([128, 4], f32)
            nc.vector.tensor_tensor(
                out=MN[:], in0=PG[:, 0:4], in1=PG[:, 4:8], op=Alu.min)
            IWH = pool.tile([128, 2], f32)
            nc.vector.tensor_tensor(
                out=IWH[:], in0=MN[:, 2:4], in1=MX[:, 0:2], op=Alu.subtract)
            EWH = pool.tile([128, 2], f32)
            nc.vector.tensor_tensor(
                out=EWH[:], in0=MX[:, 2:4], in1=MN[:, 0:2], op=Alu.subtract)
            W1 = pool.tile([128, 1], f32)
            nc.vector.tensor_scalar_max(W1[:], IWH[:, 1:2], 0.0)
            INT = pool.tile([128, 1], f32)
            nc.vector.scalar_tensor_tensor(
                out=INT[:], in0=IWH[:, 0:1], scalar=0.0, in1=W1[:],
                op0=Alu.max, op1=Alu.mult)
            ENC = pool.tile([128, 1], f32)
            nc.vector.tensor_tensor(
                out=ENC[:], in0=EWH[:, 0:1], in1=EWH[:, 1:2], op=Alu.mult)
            DWH = pool.tile([128, 4], f32)  # (pw, ph, gw, gh)
            nc.vector.tensor_tensor(
                out=DWH[:].rearrange("p (b c) -> p b c", c=2),
                in0=PG[:].rearrange("p (b c) -> p b c", c=4)[:, :, 2:4],
                in1=PG[:].rearrange("p (b c) -> p b c", c=4)[:, :, 0:2],
                op=Alu.subtract)
            A12 = pool.tile([128, 2], f32)  # (a1, a2)
            nc.vector.tensor_tensor(
                out=A12[:],
                in0=DWH[:].rearrange("p (b c) -> p b c", c=2)[:, :, 0:1],
                in1=DWH[:].rearrange("p (b c) -> p b c", c=2)[:, :, 1:2],
                op=Alu.mult)
            APA = pool.tile([128, 1], f32)
            nc.vector.tensor_tensor(
                out=APA[:], in0=A12[:, 0:1], in1=A12[:, 1:2], op=Alu.add)
            UEE = pool.tile([128, 2], f32)  # (union+eps, enc+eps)
            nc.vector.scalar_tensor_tensor(
                out=UEE[:, 0:1], in0=APA[:], scalar=EPS, in1=INT[:],
                op0=Alu.add, op1=Alu.subtract)
            nc.vector.tensor_scalar_add(UEE[:, 1:2], ENC[:], EPS)
            RUE = pool.tile([128, 2], f32)
            nc.vector.reciprocal(RUE[:], UEE[:])
            IOU = pool.tile([128, 1], f32)
            nc.vector.tensor_scalar(
                out=IOU[:], in0=INT[:], scalar1=RUE[:, 0:1], scalar2=None,
                op0=Alu.mult)
            EMU = pool.tile([128, 1], f32)  # enc - union
            nc.vector.tensor_tensor(
                out=EMU[:], in0=UEE[:, 1:2], in1=UEE[:, 0:1], op=Alu.subtract)
            Q = pool.tile([128, 1], f32)
            nc.vector.tensor_scalar(
                out=Q[:], in0=EMU[:], scalar1=RUE[:, 1:2], scalar2=None,
                op0=Alu.mult)
            GIO = pool.tile([128, 1], f32)
            nc.vector.tensor_tensor(
                out=GIO[:], in0=IOU[:], in1=Q[:], op=Alu.subtract)
            nc.vector.tensor_scalar(
                out=PART[:, 1:2], in0=GIO[:], scalar1=F[:, _PW:_PW + 1],
                scalar2=None, op0=Alu.mult)

            # ---- per-half CAM epilogue on all 128 partitions ----
            BOX = pool.tile([128, 1], f32)
            nc.vector.tensor_tensor(
                out=BOX[:], in0=ST[:, 0:1], in1=ST[:, 1:2], op=Alu.add)
            TOT = pool.tile([128, 1], f32)
            nc.vector.tensor_tensor(
                out=TOT[:], in0=ST[:, 2:3], in1=ST[:, 3:4], op=Alu.add)
            CIN = pool.tile([128, 1], f32)
            nc.vector.tensor_scalar(
                out=CIN[:], in0=BOX[:], scalar1=RR[:, 0:1], scalar2=None,
                op0=Alu.mult)
            NUM = pool.tile([128, 1], f32)
            nc.vector.tensor_tensor(
                out=NUM[:], in0=TOT[:], in1=BOX[:], op=Alu.subtract)
            COUT = pool.tile([128, 1], f32)
            nc.vector.tensor_scalar(
                out=COUT[:], in0=NUM[:], scalar1=RR[:, 1:2], scalar2=None,
                op0=Alu.mult)
            HTC = pool.tile([128, 1], f32)  # htag - cam_in
            nc.vector.tensor_tensor(
                out=HTC[:], in0=F[:, _HT:_HT + 1], in1=CIN[:], op=Alu.subtract)
            T2 = pool.tile([128, 1], f32)
            nc.vector.tensor_scalar(
                out=T2[:], in0=COUT[:], scalar1=G12[:, 1:2], scalar2=None,
                op0=Alu.mult)
            nc.vector.scalar_tensor_tensor(
                out=PART[:, 2:3], in0=HTC[:], scalar=G12[:, 0:1], in1=T2[:],
                op0=Alu.mult, op1=Alu.add)

            # ---- cross-partition reduce via PE (partials.T @ ones) ----
            PS = pp.tile([4, 1], f32)
            nc.tensor.matmul(out=PS[:], lhsT=PART[:], rhs=ONES[:],
                             start=True, stop=True)
            OS = pool.tile([4, 1], f32)
            nc.scalar.activation(out=OS[:], in_=PS[:], func=Act.Copy)
            nc.sync.dma_start(out=out.ap(), in_=OS[:])

            if debug:
                nc.sync.dma_start(out=dbg.ap()[:, 0:4], in_=PART[:])
                nc.sync.dma_start(out=dbg.ap()[:, 4:8], in_=IC[:])
                nc.sync.dma_start(out=dbg.ap()[:, 8:12], in_=ST[:])
                nc.sync.dma_start(out=dbg.ap()[:, 12:13], in_=CIN[:])
                nc.sync.dma_start(out=dbg.ap()[:, 13:14], in_=COUT[:])
                nc.sync.dma_start(out=dbg.ap()[:, 14:16], in_=SS[:])
                nc.sync.dma_start(out=dbg.ap()[:, 16:24], in_=PG[:])
                nc.sync.dma_start(out=dbg.ap()[:, 24:25], in_=GIO[:])
                nc.sync.dma_start(out=dbg.ap()[:, 25:29], in_=SC[:])
                nc.sync.dma_start(out=dbg.ap()[:, 29:33], in_=MX[:])
                nc.sync.dma_start(out=dbg.ap()[:, 33:35], in_=G12[:])

    nc.finalize()
    return nc


_NC_CACHE = {}


def _get_nc(debug=False, nslot=64):
    key = (bool(debug), int(nslot))
    if key not in _NC_CACHE:
        _NC_CACHE[key] = _build_kernel(debug=debug, nslot=nslot)
    return _NC_CACHE[key]


def make_in_maps(cam, pred_boxes, gt_boxes, gt_labels, pos_b, pos_class,
                 pos_i, pos_j, pos_gt):
    """Host-side sharding: build the per-core input maps."""
    cam = np.ascontiguousarray(np.asarray(cam, dtype=np.float32))
    pred_boxes = np.ascontiguousarray(np.asarray(pred_boxes, dtype=np.float32))
    gt_boxes = np.ascontiguousarray(np.asarray(gt_boxes, dtype=np.float32))
    gt_labels = np.asarray(gt_labels, dtype=np.int64)
    pos_b = np.asarray(pos_b, dtype=np.int64)
    pos_class = np.asarray(pos_class, dtype=np.int64)
    pos_i = np.asarray(pos_i, dtype=np.int64)
    pos_j = np.asarray(pos_j, dtype=np.int64)
    pos_gt = np.asarray(pos_gt, dtype=np.int64)

    # shared constant columns
    kvals = np.arange(1, 64, dtype=np.float32)                 # [63]
    colvals = np.arange(64, dtype=np.float32)                  # [64]
    rowvals = np.empty((128, 32), dtype=np.float32)
    rowvals[:64] = np.arange(32, dtype=np.float32)
    rowvals[64:] = np.arange(32, 64, dtype=np.float32)
    p = np.arange(128)
    pair = p % 64
    half = p // 64

    in_maps = []
    max_n = 0
    for c in range(NCORES):
        b0 = c * BPC
        cam4 = cam[b0:b0 + BPC].reshape(BPC * C * 4, QTR)
        predc = pred_boxes[b0:b0 + BPC].reshape(BPC * C * H * W, 4)
        gtbc = gt_boxes[b0:b0 + BPC].reshape(PAIRS, 4)
        glabc = gt_labels[b0:b0 + BPC].reshape(PAIRS)

        fpk = np.zeros((128, FCOLS), dtype=np.float32)
        fpk[:, _KV0:_KV0 + 63] = kvals
        fpk[:, _CV0:_CV0 + 64] = colvals
        fpk[:, _RV0:_RV0 + 32] = rowvals
        fpk[:, _GT0:_GT0 + 4] = gtbc[pair]
        fpk[:, _HT] = (p < 64).astype(np.float32)

        ipk = np.zeros((128, ICOLS), dtype=np.int32)
        # CAM: channel (b_loc*C + label) -> partition-half row 2*ch + half,
        # quarter rows 2*(2*ch + half) and +1 in cam4 [640, 1024]
        phrow = 2 * ((pair // K) * C + glabc[pair]) + half
        ipk[:, _I_CAMA] = 2 * phrow
        ipk[:, _I_CAMB] = 2 * phrow + 1

        sel = (pos_b // BPC) == c
        n = int(sel.sum())
        max_n = max(max_n, n)
        assert n <= 128, (
            f"core {c} got {n} positives; kernel pos capacity is 128")
        b_loc = pos_b[sel] - b0
        fpk[:n, _PW] = 1.0
        ipk[:n, _I_PRED] = (((b_loc * C + pos_class[sel]) * H + pos_i[sel])
                            * W + pos_j[sel])
        ipk[:n, _I_GT] = b_loc * K + pos_gt[sel]

        in_maps.append({
            "cam4": np.ascontiguousarray(cam4),
            "pred": np.ascontiguousarray(predc),
            "gtb": np.ascontiguousarray(gtbc),
            "ipk": ipk,
            "fpk": fpk,
        })
    nslot = 64 if max_n <= 64 else 128
    return in_maps, nslot


def combine_outputs(results):
    """Host-side unshard: add per-core partial sums, apply final scaling."""
    P_total = B * K  # 512 positives and 512 cam terms
    l1_sum = 0.0
    gw_sum = 0.0
    term_sum = 0.0
    for r in results:
        o = np.asarray(r["out"], dtype=np.float64).reshape(4)
        l1_sum += o[0]
        gw_sum += o[1]
        term_sum += o[2]
    loss_l1 = l1_sum / (4.0 * P_total)
    loss_giou = 1.0 - gw_sum / P_total
    loss_cam = term_sum / P_total
    loss_total = (LAMBDA_L1 * loss_l1 + LAMBDA_GIOU * loss_giou
                  + LAMBDA_CAM * loss_cam)
    return np.array([loss_l1, loss_giou, loss_cam, loss_total],
                    dtype=np.float32)


def kernel(cam, pred_boxes, gt_boxes, gt_labels, pos_b, pos_class, pos_i,
           pos_j, pos_gt, _debug=False, _trace=False):
    from concourse.bass_utils import run_bass_kernel_spmd

    in_maps, nslot = make_in_maps(cam, pred_boxes, gt_boxes, gt_labels, pos_b,
                                  pos_class, pos_i, pos_j, pos_gt)
    nc = _get_nc(debug=_debug, nslot=nslot)
    res = run_bass_kernel_spmd(nc, in_maps, core_ids=list(range(NCORES)),
                               trace=_trace)
    out = combine_outputs(res.results)
    if _debug or _trace:
        return out, res
    return out
